# revision 65
# baseline (speedup 1.0000x reference)
"""LocalTransformerEncoderLayer on 8 trn2 NeuronCores.

Sharding: core c = 2*b + h handles batch b, sequence half h (4096 tokens,
plus a 64-token halo on each side for the local-attention window).
Everything is done on-device per core; no collectives needed.

v2 pipeline (vs baseline): deeper software pipeline so the per-block
LN1 chain runs 2 blocks ahead of the FFN; window masks applied as a
0/1 multiply on expT (gpsimd) instead of rank-1 PE matmuls; transposes
batched 2-pairs-per-PSUM-tile and drained by gpsimd; fused
(av*recip)+src via scalar_tensor_tensor; x kept in bf16 only.

Layout plan (per core):
  srcT  [512, 4224] bf16  d-major haloed chunk (host-transposed)  -> QKV rhs/lhsT
  qT [128,4,4096], kT [128,4,4224] bf16 d-major in SBUF (PE: W.T @ srcT)
  v  -> DRAM scratch [33*128, 512] bf16 token-major (PE: srcT.T @ Wv)
  per q-pair p (128 query tokens, 256 keys = ext tiles p,p+1):
    simT [128keys, 2*128q] psum = kT.T @ qT;  expT = exp(scale*simT) (ACT)
    expT *= mask01 (gpsimd; window borders + halo padding, per-core data)
    den = expT.T @ ones (PE);  av [128q,512] = expT.T @ v
    s = av*(1/den) + src (DVE fused);  bn_stats/aggr -> mv
  per block b (4 pairs): batched sqrt+recip -> rstd; x_bf = (s-m)*rstd (bf16)
    PE transposes x_bf (2 pairs per [128,4,256] psum tile, gpsimd drain)
  FFN per block: h[f,tok] = relu(W1.T @ xT), y[tok,d] = h.T @ W2
  s2 = x_bf + y; LN2 token-major; DMA out fp32.
"""
import os
import numpy as np
import ml_dtypes

_BF16 = ml_dtypes.bfloat16

B, N, D, F, W = 4, 8192, 512, 2048, 64
T = N // 2            # own tokens per core = 4096
H = 64                # halo
TEXT = T + 2 * H      # 4224
NPAIR = T // 128      # 32 q-pairs per core
NBLK = T // 512       # 8 blocks
NEG = -1e10
SCALE = float(D) ** -0.5

_cache = {}


def _build(apply_bv, apply_b2, apply_ln1g, apply_ln1b, apply_ln2g, apply_ln2b):
    import concourse.bacc as bacc
    import concourse.tile as tile
    from concourse import mybir
    import concourse.bass as bass

    f32 = mybir.dt.float32
    bf16 = mybir.dt.bfloat16
    f8 = mybir.dt.float8e4
    AF = mybir.ActivationFunctionType
    ALU = mybir.AluOpType
    DR = mybir.MatmulPerfMode.DoubleRow

    nc = bacc.Bacc("TRN2", target_bir_lowering=False, debug=False)

    # ---- DRAM I/O ----
    srcT_d = nc.dram_tensor("srcT", [D, TEXT], bf16, kind="ExternalInput").ap()
    src_d = nc.dram_tensor("src", [T, D], f32, kind="ExternalInput").ap()
    wq_d = nc.dram_tensor("wq", [D, D], bf16, kind="ExternalInput").ap()
    wk_d = nc.dram_tensor("wk", [D, D], bf16, kind="ExternalInput").ap()
    wv_d = nc.dram_tensor("wv", [D, D], bf16, kind="ExternalInput").ap()
    bqT_d = nc.dram_tensor("bqT", [128, 4], f32, kind="ExternalInput").ap()
    bkT_d = nc.dram_tensor("bkT", [128, 4], f32, kind="ExternalInput").ap()
    w1_d = nc.dram_tensor("w1", [D, F], bf16, kind="ExternalInput").ap()
    b1T_d = nc.dram_tensor("b1T", [128, 16], f32, kind="ExternalInput").ap()
    w2_d = nc.dram_tensor("w2", [F, D], bf16, kind="ExternalInput").ap()
    maskA_d = nc.dram_tensor("maskA", [128, 256], f32, kind="ExternalInput").ap()
    maskB_d = nc.dram_tensor("maskB", [128, 256], f32, kind="ExternalInput").ap()
    if apply_bv or apply_b2:
        onerow_d = nc.dram_tensor("onerow", [1, 128], bf16, kind="ExternalInput").ap()
    if apply_bv:
        bvrow_d = nc.dram_tensor("bvrow", [1, D], bf16, kind="ExternalInput").ap()
    if apply_b2:
        b2row_d = nc.dram_tensor("b2row", [1, D], bf16, kind="ExternalInput").ap()
    if apply_ln1g:
        g1_d = nc.dram_tensor("g1", [128, D], f32, kind="ExternalInput").ap()
    if apply_ln1b:
        be1_d = nc.dram_tensor("be1", [128, D], f32, kind="ExternalInput").ap()
    if apply_ln2g:
        g2_d = nc.dram_tensor("g2", [128, D], f32, kind="ExternalInput").ap()
    if apply_ln2b:
        be2_d = nc.dram_tensor("be2", [128, D], f32, kind="ExternalInput").ap()
    out_d = nc.dram_tensor("out", [T, D], f32, kind="ExternalOutput").ap()
    v_d = nc.dram_tensor("vscratch", [33 * 128, D], bf16).ap()

    from contextlib import ExitStack
    with tile.TileContext(nc) as tc, ExitStack() as ctx:
        # ---- persistent pools ----
        consts = ctx.enter_context(tc.tile_pool(name="consts", bufs=1))
        kv = ctx.enter_context(tc.tile_pool(name="kv", bufs=1))
        big_ps = ctx.enter_context(tc.tile_pool(name="big_ps", bufs=2, space="PSUM"))
        av_ps = ctx.enter_context(tc.tile_pool(name="av_ps", bufs=3, space="PSUM"))
        sim_ps = ctx.enter_context(tc.tile_pool(name="sim_ps", bufs=3, space="PSUM"))

        # constants. DMA descriptor issue is serial (~0.6us each) — emission
        # order is the startup critical path: srcT block0 + wq first,
        # interleaved per-kt so the first q matmul needs only 2 descriptors.
        srcT_r0 = srcT_d.rearrange("(dt p) t -> p dt t", p=128)
        srcT0_sb = consts.tile([128, 4, 512], bf16, tag="srcT0")
        wq_sb = consts.tile([128, 4, D], bf16, tag="wq")
        wq_r = wq_d.rearrange("(kt p) m -> p kt m", p=128)
        wk_sb = consts.tile([128, 4, D], bf16, tag="wk")
        wk_r = wk_d.rearrange("(kt p) m -> p kt m", p=128)
        for kt in range(4):
            nc.sync.dma_start(srcT0_sb[:, kt:kt + 1, :], srcT_r0[:, kt:kt + 1, 0:512])
            nc.sync.dma_start(wq_sb[:, kt:kt + 1, :], wq_r[:, kt:kt + 1, :])
        for kt in range(4):
            nc.sync.dma_start(wk_sb[:, kt:kt + 1, :], wk_r[:, kt:kt + 1, :])
        bqT_sb = consts.tile([128, 4], f32, tag="bqT")
        nc.sync.dma_start(bqT_sb, bqT_d)
        bkT_sb = consts.tile([128, 4], f32, tag="bkT")
        nc.sync.dma_start(bkT_sb, bkT_d)
        wv_sb = consts.tile([128, 4, D], bf16, tag="wv")
        nc.sync.dma_start(wv_sb, wv_d.rearrange("(kt p) m -> p kt m", p=128))
        b1T_sb = consts.tile([128, 16], f32, tag="b1T")
        nc.sync.dma_start(b1T_sb, b1T_d)
        maskA_sb = consts.tile([128, 256], f32, tag="maskA")
        nc.sync.dma_start(maskA_sb, maskA_d)
        maskB_sb = consts.tile([128, 256], f32, tag="maskB")
        nc.sync.dma_start(maskB_sb, maskB_d)
        w1_sb = consts.tile([128, 4, F], bf16, tag="w1")
        w2_sb = consts.tile([128, 16, D], bf16, tag="w2")
        ones_sb = consts.tile([128, 1], bf16, tag="ones")
        nc.vector.memset(ones_sb, 1.0)
        eps_sb = consts.tile([128, 1], f32, tag="eps")
        nc.vector.memset(eps_sb, 1e-5)
        if apply_bv or apply_b2:
            onerow_sb = consts.tile([1, 128], bf16, tag="onerow")
            nc.sync.dma_start(onerow_sb, onerow_d)
        if apply_bv:
            bvrow_sb = consts.tile([1, D], bf16, tag="bvrow")
            nc.sync.dma_start(bvrow_sb, bvrow_d)
        if apply_b2:
            b2row_sb = consts.tile([1, D], bf16, tag="b2row")
            nc.sync.dma_start(b2row_sb, b2row_d)
        if apply_ln1g:
            g1_sb = consts.tile([128, D], f32, tag="g1")
            nc.sync.dma_start(g1_sb, g1_d)
        if apply_ln1b:
            be1_sb = consts.tile([128, D], f32, tag="be1")
            nc.sync.dma_start(be1_sb, be1_d)
        if apply_ln2g:
            g2_sb = consts.tile([128, D], f32, tag="g2")
            nc.sync.dma_start(g2_sb, g2_d)
        if apply_ln2b:
            be2_sb = consts.tile([128, D], f32, tag="be2")
            nc.sync.dma_start(be2_sb, be2_d)

        # persistent activations (qT: own tokens only, kT: with halo)
        qT_sb = kv.tile([128, 4, T], bf16, tag="qT")
        kT_sb = kv.tile([128, 4, TEXT], bf16, tag="kT")

        # ---- phase 1: QKV over ext grid (srcT streamed per block) ----
        srcT_r = srcT_d.rearrange("(dt p) t -> p dt t", p=128)
        with tc.tile_pool(name="srcs", bufs=3) as srcs, \
             tc.tile_pool(name="kv_io", bufs=3) as kv_io:
            blocks = [(i * 512, 512) for i in range(TEXT // 512)] + [(4096, 128)]
            eng_flip = [0]

            def drain(dst_ap, src_ap, b_col):
                # alternate psum drains between vector and scalar so neither
                # engine becomes the phase-1 serializer
                if eng_flip[0] % 2 == 0:
                    nc.vector.tensor_scalar(dst_ap, src_ap, b_col, None, ALU.add)
                else:
                    nc.scalar.activation(dst_ap, src_ap, AF.Identity, bias=b_col)
                eng_flip[0] += 1

            for off, tw in blocks:
                if off == 0:
                    srcT_sb = srcT0_sb
                else:
                    srcT_sb = srcs.tile([128, 4, 512], bf16, tag="srcT")
                    nc.sync.dma_start(srcT_sb[:, :, :tw], srcT_r[:, :, off:off + tw])
                # qT (own tokens), kT (with halo), d-major
                qlo, qhi = max(off, H), min(off + tw, H + T)
                for w_sb, b_sb, dst, lo, hi, doff in (
                    (wq_sb, bqT_sb, qT_sb, qlo, qhi, H),
                    (wk_sb, bkT_sb, kT_sb, off, off + tw, 0),
                ):
                    if lo >= hi:
                        continue
                    for dq in range(4):
                        ps = big_ps.tile([128, 512], f32, tag="big")
                        for kt in range(4):
                            nc.tensor.matmul(
                                ps[:, :tw],
                                lhsT=w_sb[:, kt, dq * 128:(dq + 1) * 128],
                                rhs=srcT_sb[:, kt, :tw],
                                start=(kt == 0), stop=(kt == 3),
                            )
                        drain(dst[:, dq, lo - doff:hi - doff],
                              ps[:, lo - off:hi - off], b_sb[:, dq:dq + 1])
                # v (token-major), per 128-token tile
                for s in range(tw // 128):
                    ti = (off + s * 128) // 128
                    ps = big_ps.tile([128, 512], f32, tag="big")
                    for kt in range(4):
                        nc.tensor.matmul(
                            ps,
                            lhsT=srcT_sb[:, kt, s * 128:s * 128 + 128],
                            rhs=wv_sb[:, kt, :],
                            start=(kt == 0), stop=(kt == 3 and not apply_bv),
                        )
                    if apply_bv:
                        nc.tensor.matmul(ps, lhsT=onerow_sb, rhs=bvrow_sb,
                                         start=False, stop=True)
                    v_t = kv_io.tile([128, D], bf16, tag="vout")
                    if eng_flip[0] % 2 == 0:
                        nc.vector.tensor_copy(v_t, ps)
                    else:
                        nc.scalar.copy(v_t, ps)
                    eng_flip[0] += 1
                    nc.sync.dma_start(v_d[ti * 128:(ti + 1) * 128, :], v_t)

        # FFN weights needed ~40us in; emit their DMAs after phase 1
        nc.sync.dma_start(w1_sb, w1_d.rearrange("(kt p) m -> p kt m", p=128))
        nc.sync.dma_start(w2_sb, w2_d.rearrange("(ft p) m -> p ft m", p=128))

        # ---- phase 2 pools ----
        xbf_pool = ctx.enter_context(tc.tile_pool(name="xbf_pool", bufs=8))
        xT_pool = ctx.enter_context(tc.tile_pool(name="xT_pool", bufs=2))
        h_pool = ctx.enter_context(tc.tile_pool(name="h_pool", bufs=1))
        io_pool = ctx.enter_context(tc.tile_pool(name="io_pool", bufs=4))
        src_pool = ctx.enter_context(tc.tile_pool(name="src_pool", bufs=4))
        stat_pool = ctx.enter_context(tc.tile_pool(name="stat_pool", bufs=4))
        mv_pool = ctx.enter_context(tc.tile_pool(name="mv_pool", bufs=12))
        vpool = ctx.enter_context(tc.tile_pool(name="vpool", bufs=5))
        exp_pool = ctx.enter_context(tc.tile_pool(name="exp_pool", bufs=4))
        s_pool = ctx.enter_context(tc.tile_pool(name="s_pool", bufs=10))
        s2_pool = ctx.enter_context(tc.tile_pool(name="s2_pool", bufs=4))

        expT_t = {}
        vpair_t = {}
        srct_t = {}
        s_t = {}
        mv8_t = {}
        xbf_t = {}
        xT_blks = {}
        h_blks = {}
        y_stash = {}

        v_r = v_d.rearrange("(t p) d -> p t d", p=128)

        def prefetch_v(p):
            vpr = vpool.tile([128, 2, D], bf16, tag="vin")
            nc.sync.dma_start(vpr, v_r[:, p:p + 2, :])
            vpair_t[p] = vpr

        def prefetch_s(p):
            srct = src_pool.tile([128, D], f32, tag="srct")
            nc.sync.dma_start(srct, src_d[p * 128:(p + 1) * 128, :])
            srct_t[p] = srct

        def emit_sim(p):
            ps_sim = sim_ps.tile([128, 256], f32, tag="sim")
            for half, ktile in ((0, p), (1, p + 1)):
                reg = ps_sim[:, half * 128:(half + 1) * 128]
                for kt in range(4):
                    nc.tensor.matmul(
                        reg,
                        lhsT=kT_sb[:, kt, ktile * 128:(ktile + 1) * 128],
                        rhs=qT_sb[:, kt, p * 128:(p + 1) * 128],
                        start=(kt == 0), stop=(kt == 3),
                    )
            # halo-padding mask (per-core data, additive -1e5 on psum)
            if p == 0:
                nc.vector.tensor_add(ps_sim, ps_sim, maskA_sb)
            elif p == NPAIR - 1:
                nc.vector.tensor_add(ps_sim, ps_sim, maskB_sb)
            expT = exp_pool.tile([128, 256], bf16, tag="expT")
            with tc.high_priority():
                nc.scalar.activation(expT, ps_sim, AF.Exp, scale=SCALE)
            # interior window mask: core-independent -> cheap memsets
            nc.gpsimd.memset(expT[0:64, 64:128], 0.0)
            nc.gpsimd.memset(expT[64:128, 128:192], 0.0)
            expT_t[p] = expT

        def emit_av(p):
            expT = expT_t.pop(p)
            vpr = vpair_t.pop(p)
            srct = srct_t.pop(p)
            ps_den = sim_ps.tile([128, 1], f32, tag="sim")
            nc.tensor.matmul(ps_den, lhsT=expT[:, 0:128], rhs=ones_sb,
                             start=True, stop=False)
            nc.tensor.matmul(ps_den, lhsT=expT[:, 128:256], rhs=ones_sb,
                             start=False, stop=True)
            recip = stat_pool.tile([128, 1], f32, tag="recip")
            with tc.high_priority():
                nc.vector.reciprocal(recip, ps_den)
            ps_av = av_ps.tile([128, 512], f32, tag="av")
            nc.tensor.matmul(ps_av, lhsT=expT[:, 0:128], rhs=vpr[:, 0, :],
                             start=True, stop=False)
            nc.tensor.matmul(ps_av, lhsT=expT[:, 128:256], rhs=vpr[:, 1, :],
                             start=False, stop=True)
            s_sb = s_pool.tile([128, D], f32, tag="s")
            with tc.high_priority():
                nc.vector.scalar_tensor_tensor(s_sb, ps_av, recip, srct,
                                               ALU.mult, ALU.add)
            s_t[p] = s_sb
            st6 = stat_pool.tile([128, 6], f32, tag="st6")
            nc.vector.bn_stats(st6, s_sb)
            blk, j = divmod(p, 4)
            if j == 0:
                mv8_t[blk] = mv_pool.tile([128, 2, 8], f32, tag="mv8",
                                          name="mv8")
            nc.vector.bn_aggr(mv8_t[blk][:, :, j], st6)

        def ln_rstd(mv8, lo, n, tag):
            """sqrt+recip over var columns mv8[:, 1, lo:lo+n] (no copies)."""
            std_blk = stat_pool.tile([128, n], f32, tag=tag + "s")
            rstd_blk = stat_pool.tile([128, n], f32, tag=tag + "r")
            with tc.high_priority():
                nc.scalar.activation(std_blk, mv8[:, 1, lo:lo + n],
                                     AF.Sqrt, bias=eps_sb)
                nc.vector.reciprocal(rstd_blk, std_blk)
            return rstd_blk

        def ln_compute(bnew):
            """One sqrt+recip for LN1(bnew) and LN2(bnew-2) combined."""
            n1 = 4 if bnew < NBLK else 0
            prev = y_stash.pop(bnew - 2, None)
            mv8 = mv8_t[bnew]
            rstd = ln_rstd(mv8, 0 if n1 else 4, n1 + (4 if prev else 0), "b1")
            for j in range(n1):
                p = bnew * 4 + j
                x_bf = xbf_pool.tile([128, D], bf16, tag="xbf")
                nc.vector.tensor_scalar(x_bf, s_t.pop(p), mv8[:, 0, j:j + 1],
                                        rstd[:, j:j + 1], ALU.subtract, ALU.mult)
                if apply_ln1g:
                    nc.vector.tensor_mul(x_bf, x_bf, g1_sb)
                if apply_ln1b:
                    nc.vector.tensor_add(x_bf, x_bf, be1_sb)
                xbf_t[p] = x_bf
            if prev:
                s2_tiles = prev
                for j in range(4):
                    p = (bnew - 2) * 4 + j
                    o_sb = io_pool.tile([128, D], f32, tag="o")
                    nc.vector.tensor_scalar(o_sb, s2_tiles[j],
                                            mv8[:, 0, 4 + j:5 + j],
                                            rstd[:, n1 + j:n1 + j + 1],
                                            ALU.subtract, ALU.mult)
                    if apply_ln2g:
                        nc.vector.tensor_mul(o_sb, o_sb, g2_sb)
                    if apply_ln2b:
                        nc.vector.tensor_add(o_sb, o_sb, be2_sb)
                    nc.sync.dma_start(out_d[p * 128:(p + 1) * 128, :], o_sb)
            mv8_t.pop(bnew, None)

        def ln2_final(blk):
            """Per-pair LN2 for the last block: j's chain completes as soon
            as its own stats do, shortening the kernel tail."""
            s2_tiles = y_stash.pop(blk)
            mv8 = mv8_t[blk + 2]
            for j in range(4):
                p = blk * 4 + j
                rstd = ln_rstd(mv8, 4 + j, 1, "fin")
                o_sb = io_pool.tile([128, D], f32, tag="o")
                nc.vector.tensor_scalar(o_sb, s2_tiles[j],
                                        mv8[:, 0, 4 + j:5 + j],
                                        rstd[:, 0:1], ALU.subtract, ALU.mult)
                if apply_ln2g:
                    nc.vector.tensor_mul(o_sb, o_sb, g2_sb)
                if apply_ln2b:
                    nc.vector.tensor_add(o_sb, o_sb, be2_sb)
                nc.sync.dma_start(out_d[p * 128:(p + 1) * 128, :], o_sb)

        def emit_transposes(blk, js):
            if js[0] == 0:
                xT_blks[blk] = xT_pool.tile([128, 4, 512], bf16, tag="xT",
                                            name="xT_blk")
            # hardware xbar transpose on the DMA path: frees the PE entirely
            for j in js:
                xbf = xbf_t[blk * 4 + j]
                with tc.high_priority():
                    nc.sync.dma_start_transpose(
                        xT_blks[blk][:, :, j * 128:(j + 1) * 128], xbf)

        def emit_ffn_h(blk):
            xT_blk = xT_blks.pop(blk)
            h_sb = h_pool.tile([128, 16, 512], bf16, tag="h")
            for ft in range(16):
                ps_h = big_ps.tile([128, 512], f32, tag="big")
                for kt in range(4):
                    nc.tensor.matmul(
                        ps_h,
                        lhsT=w1_sb[:, kt, ft * 128:(ft + 1) * 128],
                        rhs=xT_blk[:, kt, :],
                        start=(kt == 0), stop=(kt == 3),
                    )
                nc.scalar.activation(h_sb[:, ft, :], ps_h, AF.Relu,
                                     bias=b1T_sb[:, ft:ft + 1])
            h_blks[blk] = h_sb

        def emit_ffn_y(blk):
            h_sb = h_blks.pop(blk)
            s2_tiles = []
            for j in range(4):
                p = blk * 4 + j
                ps_y = big_ps.tile([128, 512], f32, tag="big")
                for ft in range(16):
                    nc.tensor.matmul(
                        ps_y,
                        lhsT=h_sb[:, ft, j * 128:(j + 1) * 128],
                        rhs=w2_sb[:, ft, :],
                        start=(ft == 0), stop=(ft == 15 and not apply_b2),
                    )
                if apply_b2:
                    nc.tensor.matmul(ps_y, lhsT=onerow_sb, rhs=b2row_sb,
                                     start=False, stop=True)
                s2 = s2_pool.tile([128, D], f32, tag="s2")
                nc.vector.tensor_add(s2, xbf_t.pop(p), ps_y)
                s2_tiles.append(s2)
                st6 = stat_pool.tile([128, 6], f32, tag="st6")
                nc.vector.bn_stats(st6, s2)
                tb = blk + 2
                if tb not in mv8_t:
                    mv8_t[tb] = mv_pool.tile([128, 2, 8], f32, tag="mv8",
                                             name="mv8")
                nc.vector.bn_aggr(mv8_t[tb][:, :, 4 + j], st6)
            y_stash[blk] = s2_tiles

        # ---- phase 2 pipeline ----
        # iteration p: sim(p) | av(p-1) | LN (early, high vector priority)
        #              | block stages (lagged 2 blocks)
        for p in range(NPAIR + 11):
            if p == 0:
                for pp in range(4):
                    prefetch_v(pp)
                for pp in range(3):
                    prefetch_s(pp)
            if p + 4 < NPAIR:
                prefetch_v(p + 4)
            if p + 3 < NPAIR:
                prefetch_s(p + 3)
            if p < NPAIR:
                emit_sim(p)
            if 1 <= p <= NPAIR:
                emit_av(p - 1)
            if p >= 6 and p % 4 == 2 and (p - 6) // 4 <= NBLK:
                ln_compute((p - 6) // 4)
            if p >= 8 and p % 4 == 0 and (p - 8) // 4 < NBLK:
                emit_transposes((p - 8) // 4, (0, 1))
            if p >= 9 and p % 4 == 1 and (p - 9) // 4 < NBLK:
                emit_transposes((p - 9) // 4, (2, 3))
            if p >= 10 and p % 4 == 2 and (p - 10) // 4 < NBLK:
                emit_ffn_h((p - 10) // 4)
            if p >= 11 and p % 4 == 3 and (p - 11) // 4 < NBLK:
                blk = (p - 11) // 4
                emit_ffn_y(blk)
                if blk == NBLK - 1:
                    ln2_final(blk)

    nc.compile()
    return nc


def _get_program(key):
    if key not in _cache:
        _cache[key] = _build(*key)
    return _cache[key]


last_exec_ns = None


def _install_ntff_hook():
    """NTFF profiling hook for axon (normally installed via antenv.axon_hooks)."""
    import sys, types
    if 'antenv.axon_hooks' in sys.modules:
        return
    mod = types.ModuleType('antenv.axon_hooks')
    _h = [None]
    mod.set_axon_ntff_profile_hook = lambda h: _h.__setitem__(0, h)
    mod.get_axon_ntff_profile_hook = lambda: _h[0]
    sys.modules['antenv.axon_hooks'] = mod
    import antenv
    antenv.axon_hooks = mod
    try:
        from trn_agent_boot.trn_boot import _ntff_profile_via_ctypes
        mod.set_axon_ntff_profile_hook(
            _ntff_profile_via_ctypes('/opt/axon/libaxon_pjrt.so'))
    except Exception:
        pass


def _build_masks(h):
    """0/1 expT masks [128 keys, 2x128 q] for pair p: keys tiles (p, p+1).

    Interior: half A (key tile p): key<64 masked for q>=64.
              half B (key tile p+1): key>=64 masked for q<64.
    Edges: h=0 p=0: keys 0:64 of tile 0 are left padding -> masked for all q.
           h=1 p=NPAIR-1: keys 64:128 of tile 32 are right padding.
    """
    mA = np.zeros((128, 256), np.float32)
    if h == 0:
        mA[0:64, 0:128] = -1e5
    mB = np.zeros((128, 256), np.float32)
    if h == 1:
        mB[64:128, 128:256] = -1e5
    return mA, mB


def kernel(src, mask, Wq, bq, Wk, bk, Wv, bv, ln1_g, ln1_b,
           W1, b1, W2, b2, ln2_g, ln2_b):
    global last_exec_ns
    src = np.asarray(src, np.float32)
    if not bool(np.asarray(mask).all()):
        raise NotImplementedError("only all-true mask supported")

    key = (bool(np.any(bv)), bool(np.any(b2)),
           not bool(np.all(ln1_g == 1)), bool(np.any(ln1_b)),
           not bool(np.all(ln2_g == 1)), bool(np.any(ln2_b)))
    nc = _get_program(key)
    apply_bv, apply_b2, a_g1, a_b1, a_g2, a_b2 = key

    shared = {
        "wq": Wq.astype(_BF16), "wk": Wk.astype(_BF16), "wv": Wv.astype(_BF16),
        "bqT": np.asarray(bq, np.float32).reshape(4, 128).T.copy(),
        "bkT": np.asarray(bk, np.float32).reshape(4, 128).T.copy(),
        "w1": W1.astype(_BF16),
        "b1T": np.asarray(b1, np.float32).reshape(16, 128).T.copy(),
        "w2": W2.astype(_BF16),
    }
    if apply_bv or apply_b2:
        shared["onerow"] = np.ones((1, 128), _BF16)
    if apply_bv:
        shared["bvrow"] = np.asarray(bv, np.float32).reshape(1, D).astype(_BF16)
    if apply_b2:
        shared["b2row"] = np.asarray(b2, np.float32).reshape(1, D).astype(_BF16)
    if a_g1:
        shared["g1"] = np.tile(np.asarray(ln1_g, np.float32).reshape(1, D), (128, 1))
    if a_b1:
        shared["be1"] = np.tile(np.asarray(ln1_b, np.float32).reshape(1, D), (128, 1))
    if a_g2:
        shared["g2"] = np.tile(np.asarray(ln2_g, np.float32).reshape(1, D), (128, 1))
    if a_b2:
        shared["be2"] = np.tile(np.asarray(ln2_b, np.float32).reshape(1, D), (128, 1))

    in_maps = []
    for c in range(8):
        b, h = divmod(c, 2)
        start = h * T - H
        ext = np.zeros((TEXT, D), np.float32)
        lo, hi = max(start, 0), min(start + TEXT, N)
        ext[lo - start: hi - start] = src[b, lo:hi]
        m = dict(shared)
        m["srcT"] = np.ascontiguousarray(ext.T).astype(_BF16)
        m["src"] = np.ascontiguousarray(src[b, h * T:(h + 1) * T])
        mA, mB = _build_masks(h)
        m["maskA"], m["maskB"] = mA, mB
        in_maps.append(m)

    from concourse.bass_utils import run_bass_kernel_spmd
    trace = bool(os.environ.get("KERNEL_TRACE"))
    if trace:
        _install_ntff_hook()
    res = run_bass_kernel_spmd(nc, in_maps, core_ids=list(range(8)), trace=trace)
    if trace:
        last_exec_ns = res.exec_time_ns

    out = np.empty((B, N, D), np.float32)
    for c in range(8):
        b, h = divmod(c, 2)
        out[b, h * T:(h + 1) * T] = res.results[c]["out"]
    return out
